# revision 67
# baseline (speedup 1.0000x reference)
"""Trainium2 Bass kernel for nn_CrossAttentionLayer (sigmoid cross-attention).

Sharding: pure data-parallel over the batch dim — core c computes batch c
(bs=8 across 8 NeuronCores, zero collectives).

Fast path (zero effective biases, which holds for the harness inputs):
  Host prep (numpy, O(bs*seq*d)):
    - LN affine folded into the projection weights; weights stored fp8
      (x16 scale so U(-1/32,1/32) entries use the fp8e4m3 normal range).
    - Per-token LN statistics (mu, 1/sqrt(var+eps)) computed on host in f32;
      shipped as small row/column vectors.  Mean subtraction is applied on
      device as rank-1 matmul corrections; rinv via per-partition scales
      (tanh scale for the k path, v-drain scale for the v path) and
      pre-scaled into the text activations for the q path.
    - Activations shipped pre-transposed (feature-major) in fp8; text as an
      (t0, t1res) fp8 residual pair, so q reaches ~bf16 effective precision
      at fp8 DoubleRow speed.  No on-device transposes or layout copies.
    - Exact 0.5*colsum(v) row (sv): the out einsum is restructured as
      out = 0.5*sv + sum_kv (attn-0.5)*v using tanh(s/2) = 2*sigmoid(s)-1,
      so fp8 v-quantization noise is only weighted by the zero-mean
      (attn-0.5) factor and the dominant systematic error term is computed
      exactly on host.

  Device program per core (batch-local shapes: text (512,1024), av (1024,1024)):
    - q/k/v projections as fp8 DoubleRow matmuls (two K=128 contraction
      chunks per instruction -> 0.5 cycles/row); q additionally consumes the
      (t0, t1res) residual pair against a stride-0-broadcast weight subtile.
    - Per-head-pair attention: scores via fp8 DoubleRow with broadcast k
      against the (q0, q1res) fp8 residual pair of q (built from the q PSUM
      by a copy + subtract), tanh on ACT with per-partition rinv_a scale,
      out einsum in bf16 (kv-contraction, M=128) + rank-1 sv term.
    - The v projection runs as PE filler woven through the first head-pairs
      in a single rotating PSUM bank; k projection per kv-half in one bank;
      8 PSUM banks fully allocated (scores 2x2 + k 1 + q/out 2 + v 1).
    - attn-mean over heads via bf16 pairwise add tree balanced across
      DVE/GpSimd, final transpose via raw PE matmul against ident/32 (folds
      the 1/16 head-mean and tanh->sigmoid 0.5 scales), +0.5 on the drain,
      am stored bf16 (host converts to f32).
    - Scheduling: column-sliced weight DMAs ordered by first use, merged
      small-row params (one HWDGE slot each), ACT table pre-warm, PE
      p-state warmup matmuls, fb0 copies on the then-idle ACT, projection
      thunks paced 4/score with the k-jh1 chain spilled across head-pair
      boundaries.
"""
import numpy as np
import ml_dtypes

import concourse.bacc as bacc
import concourse.mybir as mybir
import concourse.tile as tile
from concourse.bass_utils import run_bass_kernel_spmd

bf16 = ml_dtypes.bfloat16
fp8 = ml_dtypes.float8_e4m3
BF = mybir.dt.bfloat16
FP8 = mybir.dt.float8e4
F32 = mybir.dt.float32
AF = mybir.ActivationFunctionType
ALU = mybir.AluOpType
DR = mybir.MatmulPerfMode.DoubleRow

NW = 512      # num_word (queries)
NV = 1024     # num_valid (keys/values)
D = 1024      # d_model
H = 16        # heads
DK = 64       # head dim
NCORES = 8
WSCALE = 16.0
LN_EPS = 1e-5

# fraction of attn-mean add rows assigned to DVE (rest on GpSimd)
DVE_ADD_SHARE = 0.56

_CACHE: dict = {}


def _build_program_fp8():
    nc = bacc.Bacc("TRN2", target_bir_lowering=False, debug=False)

    aT_d = nc.declare_dram_parameter("aT8", [D, NV], FP8, isOutput=False)
    # text ships as (t0, t1res) fp8 residual pairs per feature chunk
    tT_d = nc.declare_dram_parameter("tT8", [D, 2 * NW], FP8, isOutput=False)
    wq_d = nc.declare_dram_parameter("wq8", [D, D], FP8, isOutput=False)
    wk_d = nc.declare_dram_parameter("wk8", [D, D], FP8, isOutput=False)
    wv_d = nc.declare_dram_parameter("wv8", [D, D], FP8, isOutput=False)
    # merged small rows: qrow = [mrt | wsq], krow = [mra | wsk],
    # vrow = [wsv | svt]; scales = [sca | rva]
    qrow_d = nc.declare_dram_parameter("qrow", [1, NW + D], BF, isOutput=False)
    krow_d = nc.declare_dram_parameter("krow", [1, NV + D], BF, isOutput=False)
    vrow_d = nc.declare_dram_parameter("vrow", [1, D + 8 * 512], BF,
                                       isOutput=False)
    scl_d = nc.declare_dram_parameter("scl", [128, 16], F32, isOutput=False)
    id16_d = nc.declare_dram_parameter("id16", [128, 128], BF, isOutput=False)

    out_d = nc.declare_dram_parameter("out", [NW, D], F32, isOutput=True)
    am_d = nc.declare_dram_parameter("am", [NW, NV], BF, isOutput=True)

    with tile.TileContext(nc) as tc:
        import contextlib
        with contextlib.ExitStack() as ctx:
            const_p = ctx.enter_context(tc.tile_pool(name="const", bufs=1))
            w_p = ctx.enter_context(tc.tile_pool(name="w", bufs=1))
            xT_p = ctx.enter_context(tc.tile_pool(name="xT", bufs=1))
            v_p = ctx.enter_context(tc.tile_pool(name="v", bufs=1))

            # feature-major fp8 activations, straight from DMA
            aT = xT_p.tile([128, 8 * NV], FP8, name="aT")   # [p, db*1024 + j]
            tT = xT_p.tile([128, 16 * NW], FP8, name="tT")  # [p, db, s, t]
            wv_sb = w_p.tile([128, 8 * D], FP8, name="wv_sb")
            wq_sb = w_p.tile([128, 8 * D], FP8, name="wq_sb")
            wk_sb = w_p.tile([128, 8 * D], FP8, name="wk_sb")

            # db-pair views for DoubleRow contraction
            aT_r = aT[:, :].rearrange("p (g c) -> p g c", c=NV)
            tT_r = tT[:, :].rearrange("p (g s c) -> p g s c", g=8, s=2)
            wv_r = wv_sb[:, :].rearrange("p (g c) -> p g c", c=D)
            wq_r = wq_sb[:, :].rearrange("p (g c) -> p g c", c=D)
            wk_r = wk_sb[:, :].rearrange("p (g c) -> p g c", c=D)

            # --- DMA schedule, ordered by first use: fb0's k projection
            # (wk fb0/1 column slice + kv-half of aT), then the q path, then
            # the v path, then the steady-state remainders.  Weight loads are
            # column-sliced so fb0 work starts ~2us in.
            def wslice(dst_r, dram, c0, c1):
                nc.sync.dma_start(
                    dst_r[:, :, c0:c1],
                    dram[:, c0:c1].rearrange("(g p) d -> p g d", p=128))

            tT_src = tT_d[:, :].rearrange("(g p) t -> p g t", p=128)
            nc.sync.dma_start(tT[:, 0:4 * 2 * NW], tT_src[:, 0:4, :])
            nc.sync.dma_start(tT[:, 4 * 2 * NW:], tT_src[:, 4:, :])
            wslice(wq_r, wq_d, 0, 128)
            qrow_sb = const_p.tile([1, NW + D], BF, name="qrow_sb")
            nc.sync.dma_start(qrow_sb[:], qrow_d[:])
            mrt_sb = qrow_sb[0:1, 0:NW]
            wsq_sb = qrow_sb[0:1, NW:]
            wslice(wk_r, wk_d, 0, 128)
            krow_sb = const_p.tile([1, NV + D], BF, name="krow_sb")
            nc.sync.dma_start(krow_sb[:], krow_d[:])
            mra_sb = krow_sb[0:1, 0:NV]
            wsk_sb = krow_sb[0:1, NV:]
            nc.sync.dma_start(
                aT_r[:, :, 0:512],
                aT_d[:, 0:512].rearrange("(g p) t -> p g t", p=128))
            wslice(wk_r, wk_d, 128, 256)
            wslice(wq_r, wq_d, 128, 256)
            scl_sb = const_p.tile([128, 16], F32, name="scl_sb")
            nc.sync.dma_start(scl_sb[:], scl_d[:])
            sca_sb = scl_sb[:, 0:8]
            rva_sb = scl_sb[:, 8:16]
            nc.sync.dma_start(
                aT_r[:, :, 512:1024],
                aT_d[:, 512:1024].rearrange("(g p) t -> p g t", p=128))
            wslice(wv_r, wv_d, 0, 512)
            vrow_sb = const_p.tile([1, D + 8 * 512], BF, name="vrow_sb")
            nc.sync.dma_start(vrow_sb[:], vrow_d[:])
            wsv_sb = vrow_sb[0:1, 0:D]
            svt_sb = vrow_sb[0:1, D:]
            wslice(wk_r, wk_d, 256, 1024)
            wslice(wq_r, wq_d, 256, 1024)
            wslice(wv_r, wv_d, 512, 1024)
            id16 = const_p.tile([128, 128], BF, name="id16")
            nc.sync.dma_start(id16[:], id16_d[:])
            ones11 = const_p.tile([1, 128], BF, name="ones11")
            nc.gpsimd.memset(ones11[:], 1.0)
            half128 = const_p.tile([128, 1], F32, name="half128")
            nc.gpsimd.memset(half128[:], 0.5)
            # warm the ACT function tables during the DMA wait so the
            # 1.3us table load is off the first-sigmoid critical path
            warm = const_p.tile([1, 2], F32, name="warm")
            nc.gpsimd.memset(warm[:], 0.0)
            nc.scalar.activation(warm[0:1, 0:1], warm[0:1, 0:1], AF.Tanh)
            nc.scalar.activation(warm[0:1, 1:2], warm[0:1, 1:2], AF.Identity,
                                 bias=0.0)

            # fp8 q/k rings; qt holds (q0, q1res) fp8 residual pairs
            qt_ring = [xT_p.tile([128, 2 * NW], FP8, name=f"qtr{i}")
                       for i in range(3)]
            kt_ring = [xT_p.tile([128, NV], FP8, name=f"ktr{i}")
                       for i in range(3)]

            v = [v_p.tile([128, D], BF, name=f"v{jb}") for jb in range(8)]

            # PSUM plan (8 banks): scores 2x[128,1024] (4) + k-proj per-jh
            # [128,512] (1) + {qps, out, tail-transpose} 2x[128,512] (2) +
            # v-proj rotating [128,512] (1).  The v projection runs as PE
            # filler woven into the per-head-pair loop instead of a
            # standalone phase, so the sigmoid pipeline starts ~7us in.
            with (
                tc.tile_pool(name="sp_ps", bufs=2, space="PSUM") as sp_ps,
                tc.tile_pool(name="kq_ps", bufs=1, space="PSUM") as kq_ps,
                tc.tile_pool(name="vp2_ps", bufs=2, space="PSUM") as vp2_ps,
                tc.tile_pool(name="vv_ps", bufs=1, space="PSUM") as vv_ps,
            ):
                o_ps = vp2_ps

                def v_thunks():
                    # one thunk per (fh, jb): 4 db-pair DoubleRow matmuls +
                    # rank-1 mean fixup + DVE drain with rinv_a/WSCALE/2 scale
                    for fh in range(2):
                        for jb in range(8):
                            def mm(fh=fh, jb=jb):
                                ps = vv_ps.tile([128, 512], F32, tag="vps",
                                                name=f"vps{fh}_{jb}")
                                for dp in range(4):
                                    nc.tensor.matmul(
                                        ps[:],
                                        aT_r[:, 2 * dp:2 * dp + 2,
                                             jb * 128:(jb + 1) * 128],
                                        wv_r[:, 2 * dp:2 * dp + 2,
                                             fh * 512:(fh + 1) * 512],
                                        start=(dp == 0), stop=False,
                                        perf_mode=DR)
                                nc.tensor.matmul(
                                    ps[:],
                                    mra_sb[0:1, jb * 128:(jb + 1) * 128],
                                    wsv_sb[0:1, fh * 512:(fh + 1) * 512],
                                    start=False, stop=True)
                                nc.vector.tensor_scalar_mul(
                                    v[jb][:, fh * 512:(fh + 1) * 512], ps[:],
                                    rva_sb[:, jb:jb + 1])
                            yield mm

                vfill = v_thunks()
                vleft = [16]

                # PE p-state warmup: dummy matmuls bridge the DMA wait so
                # the first real projections run at full clock (the ramp
                # model needs ~3us of continuous PE busy)
                wrow = const_p.tile([1, 512], BF, name="wrow")
                nc.gpsimd.memset(wrow[:], 0.0)
                wps = vv_ps.tile([128, 512], F32, tag="vps", name="wps")
                for i in range(6):
                    nc.tensor.matmul(wps[:], ones11[0:1, :], wrow[0:1, :],
                                     start=True, stop=True,
                                     skip_group_check=True)
                pt_p = ctx.enter_context(tc.tile_pool(name="pt", bufs=8))
                s1_p = ctx.enter_context(tc.tile_pool(name="s1", bufs=12))
                tree_p = ctx.enter_context(tc.tile_pool(name="tree", bufs=12))
                row_p = ctx.enter_context(tc.tile_pool(name="row", bufs=1))
                oc_p = ctx.enter_context(tc.tile_pool(name="oc", bufs=4))

                am_rows = row_p.tile([128, 8 * 512], BF, name="am_rows")

                pend = [None] * 4
                acc = [None] * 4
                # weighted DVE/GpSimd balance for the attn-mean adds
                addrows = [0, 0]   # [dve, pool]

                gi_now = [0]

                def bal_add(dst, a, b, rows, dve=False):
                    tot = addrows[0] + addrows[1] + rows
                    # early jbs: keep DVE clear for v drains + q/k copies;
                    # late-fb boundary jbs: keep DVE clear for the next fb's
                    # k/q PSUM drains (FIFO order would delay them)
                    g = gi_now[0]
                    use_dve = dve or (18 <= g and
                                      addrows[0] + rows <= DVE_ADD_SHARE * tot)
                    if use_dve:
                        eng = nc.vector
                        addrows[0] += rows
                    else:
                        eng = nc.gpsimd
                        addrows[1] += rows
                    eng.tensor_add(dst, a, b)

                def mean_insert(jp, c, fb):
                    # pair tree through fb5, then direct folds: fb6's fold
                    # runs during fb7's compute, leaving exactly one add on
                    # the post-fb7 critical tail
                    if fb < 6:
                        if pend[jp] is None:
                            pend[jp] = c
                            return
                        pr = pend[jp]
                        pend[jp] = None
                        nt = tree_p.tile([128, NV], BF, tag="tree")
                        bal_add(nt[:], pr[:], c[:], 1024)
                        if acc[jp] is None:
                            acc[jp] = nt
                        else:
                            na = tree_p.tile([128, NV], BF, tag="tree")
                            bal_add(na[:], acc[jp][:], nt[:], 1024)
                            acc[jp] = na
                    else:
                        na = tree_p.tile([128, NV], BF, tag="tree")
                        bal_add(na[:], acc[jp][:], c[:], 1024, dve=(fb == 7))
                        acc[jp] = na

                def qk_thunks(fb):
                    """Deferred next-fb projection matmuls: q first (its copy
                    clears DVE early), then k; interleaved between score
                    batches so ACT is never starved at an fb boundary."""
                    qps = vp2_ps.tile([128, NW], F32, tag="vp2",
                                      name=f"qps{fb}")
                    qt = qt_ring[fb % 3]
                    kt = kt_ring[fb % 3]
                    thunks = []
                    for db in range(8):
                        # (t0 + t1res) residual pair against broadcast weight
                        thunks.append(lambda db=db: nc.tensor.matmul(
                            qps[:],
                            wq_r[:, db, fb * 128:(fb + 1) * 128]
                            .unsqueeze(1).broadcast_to([128, 2, 128]),
                            tT_r[:, db, :, :],
                            start=(db == 0), stop=False, perf_mode=DR))

                    def qfin():
                        nc.tensor.matmul(
                            qps[:], wsq_sb[0:1, fb * 128:(fb + 1) * 128],
                            mrt_sb[0:1, :], start=False, stop=True)
                        if fb == 0:
                            # ACT is idle before its first tanh: use it for
                            # fb0's copy so the DVE chain is shorter
                            nc.scalar.copy(qt[:, 0:NW], qps[:])
                        else:
                            nc.vector.tensor_copy(qt[:, 0:NW], qps[:])
                        # fp8 residual plane: q1 = qps - fp8(qps)
                        nc.vector.tensor_sub(qt[:, NW:], qps[:], qt[:, 0:NW])
                    thunks.append(qfin)

                    # k projection per kv-half in a single rotating PSUM bank
                    kctx = {}

                    def kj(jh, dp):
                        if dp == 0:
                            # late fbs alternate into the vv slot (free once
                            # the v projection drains), so consecutive fbs
                            # never WAR on the same k PSUM bank
                            if fb >= 5 and fb % 2 == 1:
                                kctx["ps"] = vv_ps.tile(
                                    [128, 512], F32, tag="vps",
                                    name=f"kpsv{fb}_{jh}")
                            else:
                                kctx["ps"] = kq_ps.tile(
                                    [128, 512], F32, tag="kps",
                                    name=f"kps{fb}_{jh}")
                        nc.tensor.matmul(
                            kctx["ps"][:],
                            wk_r[:, 2 * dp:2 * dp + 2,
                                 fb * 128:(fb + 1) * 128],
                            aT_r[:, 2 * dp:2 * dp + 2,
                                 jh * 512:(jh + 1) * 512],
                            start=(dp == 0), stop=False, perf_mode=DR)

                    def kfin(jh):
                        nc.tensor.matmul(
                            kctx["ps"][:],
                            wsk_sb[0:1, fb * 128:(fb + 1) * 128],
                            mra_sb[0:1, jh * 512:(jh + 1) * 512],
                            start=False, stop=True)
                        if fb == 0 and jh == 0:
                            nc.scalar.copy(kt[:, 0:512], kctx["ps"][:])
                        else:
                            nc.vector.tensor_copy(
                                kt[:, jh * 512:(jh + 1) * 512], kctx["ps"][:])
                    for jh in range(2):
                        for dp in range(4):
                            thunks.append(lambda jh=jh, dp=dp: kj(jh, dp))
                        thunks.append(lambda jh=jh: kfin(jh))
                    return qt, kt, thunks

                # ------------- fused per-head-pair (fb) loop -------------
                # Flat (fb, jb) pipeline: out matmuls run two kv-blocks
                # behind the sigmoids and straddle fb boundaries, so at a
                # boundary the next fb's scores are issued before the last
                # out matmuls of the previous fb (neither engine waits on
                # the other's tail).
                qt_next, kt_next, th = qk_thunks(0)
                for t in th[:14]:
                    t()
                carry = th[14:]

                fbctx = {}
                outq = []

                def pop_out():
                    fb2, jb2, pt2 = outq.pop(0)
                    c = fbctx[fb2]
                    if c["ops"] is None:
                        c["ops"] = o_ps.tile([128, 512], F32, tag="vp2",
                                             name=f"ops{fb2}")
                    o_ps_t = c["ops"]
                    h0, h1 = 2 * fb2, 2 * fb2 + 1
                    for h in range(2):
                        hh = (h0 if h == 0 else h1)
                        for qb in range(4):
                            nc.tensor.matmul(
                                o_ps_t[:, qb * 128 + h * 64:
                                       qb * 128 + h * 64 + 64],
                                pt2[:, h * 512 + qb * 128:
                                    h * 512 + (qb + 1) * 128],
                                v[jb2][:, hh * 64:(hh + 1) * 64],
                                start=(jb2 == 0 and h == 0 and qb == 0),
                                stop=(jb2 == 7 and h == 1 and qb == 3),
                                skip_group_check=True)
                    if jb2 == 0:
                        # exact 0.5*colsum(v) term: out = 0.5*sv + sum
                        # (attn-0.5)*v, so fp8 v-quantization noise is only
                        # weighted by the zero-mean (attn-0.5) factor
                        nc.tensor.matmul(
                            o_ps_t[:], ones11[0:1, :],
                            svt_sb[0:1, fb2 * 512:(fb2 + 1) * 512],
                            start=False, stop=False, skip_group_check=True)
                    if jb2 == 7:
                        # out_ps -> f32 staging -> per-fb column-slice store
                        oc = oc_p.tile([128, 512], F32, tag="oc")
                        nc.vector.tensor_copy(oc[:], o_ps_t[:])
                        nc.sync.dma_start(
                            out_d[:, :].rearrange(
                                "(qb p) d -> p qb d",
                                p=128)[:, :, fb2 * 128:(fb2 + 1) * 128],
                            oc[:])
                        del fbctx[fb2]

                qt = kt = None
                qtv = None
                thunks = []
                for gi in range(72):
                    gi_now[0] = gi
                    fb, jb = divmod(gi, 8) if gi < 64 else (8, 0)
                    if gi < 64:
                        if jb == 0:
                            qt, kt = qt_next, kt_next
                            qtv = qt[:, :].rearrange("p (s c) -> p s c", s=2)
                            thunks = []
                            if fb < 7:
                                qt_next, kt_next, thunks = qk_thunks(fb + 1)
                            thunks = carry + thunks
                            carry = []
                            fbctx[fb] = {
                                "ops": None,
                                "s1s": [s1_p.tile([128, NV], BF, tag="s1",
                                                  name=f"s1f_{fb}_{i}")
                                        for i in range(4)],
                            }
                        # scores S^T[kv, q] for both heads of the pair;
                        # fp8 DoubleRow: broadcast k against the (q0, q1res)
                        # residual pair, so q reaches ~bf16 effective precision
                        # at fp8 DoubleRow speed.
                        sp = sp_ps.tile([128, NV], F32, tag="sp")
                        for h in range(2):
                            nc.tensor.matmul(
                                sp[:, h * 512:(h + 1) * 512],
                                kt[h * 64:(h + 1) * 64,
                                   jb * 128:(jb + 1) * 128]
                                .unsqueeze(1).broadcast_to([64, 2, 128]),
                                qtv[h * 64:(h + 1) * 64, :, :],
                                start=True, stop=True, perf_mode=DR)
                        # pt = tanh(s/2) = 2*sigmoid(s) - 1, i.e. 2*(attn-0.5)
                        pt = pt_p.tile([128, NV], BF, tag="pt")
                        nc.scalar.activation(pt[:], sp[:], AF.Tanh,
                                             scale=sca_sb[:, jb:jb + 1])
                        outq.append((fb, jb, pt))
                        # weave in next-fb projection matmuls + v-proj filler
                        for _ in range(4):
                            if thunks:
                                thunks.pop(0)()
                        # v pacing: fh0 drains 1/jb (gi1-8), fh1 spread over
                        # every other jb afterwards, so the in-order DVE queue
                        # never backs up behind a burst of v drains
                        if vleft[0] > 0 and (3 <= gi <= 10 or
                                             (gi >= 12 and gi % 2 == 0)):
                            next(vfill)()
                            vleft[0] -= 1
                        if jb == 7:
                            # the next fb's k-jh1 chain isn't needed until its
                            # jb4 scores: spill it across the boundary instead
                            # of flushing it ahead of the next fb's scores
                            while len(thunks) > 5:
                                thunks.pop(0)()
                            carry = thunks
                            thunks = []
                    if outq and (len(outq) > (4 if gi < 10 else 2)
                                 or gi >= 64):
                        pop_out()
                    if gi < 64:
                        # head-pair partial sum for attn-mean
                        jp, half = jb // 2, jb % 2
                        bal_add(fbctx[fb]["s1s"][jp][:, half * 512:
                                                     (half + 1) * 512],
                                pt[:, 0:512], pt[:, 512:1024], 512)
                        if jb == 7:
                            for jp in range(4):
                                mean_insert(jp, fbctx[fb]["s1s"][jp], fb)

                # ---------------- attn-mean finalization ----------------
                # transpose via raw matmul against ident/32: folds the
                # 1/16 head-mean and the tanh->sigmoid 0.5 scale into the
                # transpose (real matmul, so the scaled identity is honored
                # numerically); the sigmoid +0.5 offset rides on the drain
                am_dst = am_d[:, :].rearrange(
                    "(qb p) (grp c) -> p qb grp c", p=128, c=512)
                for si, sec in enumerate((0, 2, 4, 6, 1, 3, 5, 7)):
                    qb, grp = divmod(sec, 2)
                    tp = o_ps.tile([128, 512], F32, tag="vp2")
                    for j in range(4):
                        jb = grp * 4 + j
                        jp, half = jb // 2, jb % 2
                        nc.tensor.matmul(
                            tp[:, j * 128:(j + 1) * 128],
                            acc[jp][:, half * 512 + qb * 128:
                                    half * 512 + (qb + 1) * 128],
                            id16[:],
                            start=(j == 0), stop=(j == 3),
                            skip_group_check=True)
                    # drains alternate ACT/DVE so the tail chain pipelines
                    amr = am_rows[:, sec * 512:(sec + 1) * 512]
                    if si % 2 == 0:
                        nc.scalar.activation(amr, tp[:], AF.Identity,
                                             bias=half128[:])
                    else:
                        nc.vector.tensor_scalar_add(amr, tp[:], half128[:])
                    qb2, grp2 = divmod(sec, 2)
                    nc.sync.dma_start(
                        am_dst[:, qb2:qb2 + 1, grp2, :],
                        amr.unsqueeze(1))

    nc.compile()
    return nc


def _build_program(with_v_bias: bool):
    """General fallback: bf16, LN computed on device (handles any biases)."""
    nc = bacc.Bacc("TRN2", target_bir_lowering=False, debug=False)

    xt_d = nc.declare_dram_parameter("xt", [NW, D], BF, isOutput=False)
    xa_d = nc.declare_dram_parameter("xa", [NV, D], BF, isOutput=False)
    wq_d = nc.declare_dram_parameter("wqT", [D, D], BF, isOutput=False)
    wk_d = nc.declare_dram_parameter("wkT", [D, D], BF, isOutput=False)
    wv_d = nc.declare_dram_parameter("wvT", [D, D], BF, isOutput=False)
    bq_d = nc.declare_dram_parameter("bq", [D], F32, isOutput=False)
    bk_d = nc.declare_dram_parameter("bk", [D], F32, isOutput=False)
    bv_d = nc.declare_dram_parameter("bv", [1, D], BF, isOutput=False)
    id_d = nc.declare_dram_parameter("ident", [128, 128], BF, isOutput=False)

    out_d = nc.declare_dram_parameter("out", [NW, D], F32, isOutput=True)
    am_d = nc.declare_dram_parameter("am", [NW, NV], F32, isOutput=True)

    that_dram = nc.dram_tensor("that_scratch", [NW, D], BF)
    ahat_dram = nc.dram_tensor("ahat_scratch", [NV, D], BF)

    with tile.TileContext(nc) as tc:
        import contextlib
        with contextlib.ExitStack() as ctx:
            const_p = ctx.enter_context(tc.tile_pool(name="const", bufs=1))
            w_p = ctx.enter_context(tc.tile_pool(name="w", bufs=1))
            xT_p = ctx.enter_context(tc.tile_pool(name="xT", bufs=1))
            v_p = ctx.enter_context(tc.tile_pool(name="v", bufs=1))
            stat_p = ctx.enter_context(tc.tile_pool(name="stat", bufs=12))

            eps_t = const_p.tile([128, 1], F32)
            nc.gpsimd.memset(eps_t[:], 1e-5)

            # feature-major normalized activations
            tT = xT_p.tile([128, 8 * NW], BF, name="tT")   # [p, fb*512 + t]
            aT = xT_p.tile([128, 8 * NV], BF, name="aT")   # [p, db*1024 + j]

            # ------------- Phase 1: LayerNorm (av stream first) -------------
            apply_alt = [0]

            def layer_norm(src_dram, dst_dram, xin, ngroups, xT_tile, ncols):
                half = ngroups // 2
                src = src_dram[:, :].rearrange("(g p) d -> p g d", p=128)
                nc.sync.dma_start(xin[:, 0:half * D], src[:, 0:half, :])
                nc.sync.dma_start(xin[:, half * D:], src[:, half:, :])
                for g in range(ngroups):
                    sl = slice(g * D, (g + 1) * D)
                    st = stat_p.tile([128, 12], F32, tag="st12")
                    nc.vector.bn_stats(st[:, 0:6], xin[:, g * D:g * D + 512])
                    nc.vector.bn_stats(st[:, 6:12], xin[:, g * D + 512:(g + 1) * D])
                    mv = stat_p.tile([128, 2], F32, tag="mv")
                    nc.vector.bn_aggr(mv[:], st[:])
                    std = stat_p.tile([128, 1], F32, tag="std")
                    nc.scalar.activation(std[:], mv[:, 1:2], AF.Sqrt, bias=eps_t[:])
                    rinv = stat_p.tile([128, 1], F32, tag="rinv")
                    nc.vector.reciprocal(rinv[:], std[:])
                    eng = nc.vector if apply_alt[0] % 2 == 0 else nc.gpsimd
                    apply_alt[0] += 1
                    eng.tensor_scalar(
                        xin[:, sl], xin[:, sl], mv[:, 0:1], rinv[:],
                        ALU.subtract, ALU.mult)
                nc.sync.dma_start(
                    dst_dram[:, :].rearrange("(g p) d -> p g d", p=128), xin[:])
                for db in range(8):
                    nc.sync.dma_start(
                        xT_tile[:, db * ncols:(db + 1) * ncols],
                        dst_dram[:, db * 128:(db + 1) * 128], transpose=True)

            with tc.tile_pool(name="ln", bufs=1) as ln_p:
                xa_sb = ln_p.tile([128, 8 * D], BF, name="xa_sb")
                xt_sb = ln_p.tile([128, 4 * D], BF, name="xt_sb")
                layer_norm(xa_d, ahat_dram, xa_sb, 8, aT, NV)
                layer_norm(xt_d, that_dram, xt_sb, 4, tT, NW)

            ident = const_p.tile([128, 128], BF)
            nc.sync.dma_start(ident[:], id_d[:])
            bq_sb = const_p.tile([128, 8], F32)
            nc.sync.dma_start(bq_sb[:], bq_d[:].rearrange("(a p) -> p a", p=128))
            bk_sb = const_p.tile([128, 8], F32)
            nc.sync.dma_start(bk_sb[:], bk_d[:].rearrange("(a p) -> p a", p=128))
            if with_v_bias:
                bv_sb = const_p.tile([1, D], BF)
                nc.sync.dma_start(bv_sb[:], bv_d[:])
                ones_t = const_p.tile([1, 128], BF)
                nc.gpsimd.memset(ones_t[:], 1.0)

            def load_w(dram):
                t = w_p.tile([128, 8 * D], BF, name=f"w_{dram.name}")
                src = dram[:, :].rearrange("(a p) d -> p a d", p=128)
                nc.sync.dma_start(t[:, 0:4 * D], src[:, 0:4, :])
                nc.sync.dma_start(t[:, 4 * D:], src[:, 4:, :])
                return t

            wv_sb = load_w(wv_d)
            wq_sb = load_w(wq_d)
            wk_sb = load_w(wk_d)

            qt_p = ctx.enter_context(tc.tile_pool(name="qt", bufs=8))
            kt_p = ctx.enter_context(tc.tile_pool(name="kt", bufs=3))
            pt_p = ctx.enter_context(tc.tile_pool(name="pt", bufs=8))
            s1_p = ctx.enter_context(tc.tile_pool(name="s1", bufs=12))
            tree_p = ctx.enter_context(tc.tile_pool(name="tree", bufs=12))
            fin_p = ctx.enter_context(tc.tile_pool(name="fin", bufs=4))
            row_p = ctx.enter_context(tc.tile_pool(name="row", bufs=1))
            oc_p = ctx.enter_context(tc.tile_pool(name="oc", bufs=4))

            # ------- Phase 2: v projection (db-outer) + q projections -------
            v = [v_p.tile([128, D], BF, name=f"v{jb}") for jb in range(8)]
            with tc.tile_pool(name="vp_ps", bufs=8, space="PSUM") as vp_ps:
                for fh in range(2):
                    pss = [vp_ps.tile([128, 512], F32, tag="vp",
                                      name=f"vps{fh}_{jb}") for jb in range(8)]
                    for db in range(8):
                        for jb in range(8):
                            nc.tensor.matmul(
                                pss[jb][:],
                                aT[:, db * NV + jb * 128:db * NV + (jb + 1) * 128],
                                wv_sb[:, db * D + fh * 512:db * D + (fh + 1) * 512],
                                start=(db == 0), stop=(db == 7 and not with_v_bias))
                    if with_v_bias:
                        for jb in range(8):
                            nc.tensor.matmul(
                                pss[jb][:], ones_t[:],
                                bv_sb[0:1, fh * 512:(fh + 1) * 512],
                                start=False, stop=True)
                    for jb in range(8):
                        nc.vector.tensor_copy(v[jb][:, fh * 512:(fh + 1) * 512],
                                              pss[jb][:])

                # q projections for all feature blocks
                qts = []
                for fb in range(8):
                    ps = vp_ps.tile([128, NW], F32, tag="vp")
                    for db in range(8):
                        nc.tensor.matmul(
                            ps[:], wq_sb[:, db * D + fb * 128:db * D + (fb + 1) * 128],
                            tT[:, db * NW:(db + 1) * NW],
                            start=(db == 0), stop=(db == 7))
                    qt = qt_p.tile([128, NW], BF, tag="qt")
                    nc.vector.tensor_scalar_add(qt[:], ps[:], bq_sb[:, fb:fb + 1])
                    qts.append(qt)

            with (
                tc.tile_pool(name="sp_ps", bufs=3, space="PSUM") as sp_ps,
                tc.tile_pool(name="o_ps", bufs=2, space="PSUM") as o_ps,
            ):
                am_row = [row_p.tile([128, NV], BF, name=f"am_row{i}")
                          for i in range(4)]

                pend = [None] * 4
                acc = [None] * 4
                alt = [0]

                def bal_add(dst, a, b, dve=False):
                    eng = nc.vector if (dve or alt[0] % 2 == 1) else nc.gpsimd
                    alt[0] += 1
                    eng.tensor_add(dst, a, b)

                def mean_insert(jp, c, last):
                    if pend[jp] is None:
                        pend[jp] = c
                        return
                    pr = pend[jp]
                    pend[jp] = None
                    nt = tree_p.tile([128, NV], BF, tag="tree")
                    bal_add(nt[:], pr[:], c[:], dve=last)
                    if acc[jp] is None:
                        acc[jp] = nt
                    else:
                        na = tree_p.tile([128, NV], BF, tag="tree")
                        bal_add(na[:], acc[jp][:], nt[:], dve=last)
                        acc[jp] = na

                def k_proj(fb):
                    kps = sp_ps.tile([128, NV], F32, tag="sp")
                    for jh in range(2):
                        for db in range(8):
                            nc.tensor.matmul(
                                kps[:, jh * 512:(jh + 1) * 512],
                                wk_sb[:, db * D + fb * 128:db * D + (fb + 1) * 128],
                                aT[:, db * NV + jh * 512:db * NV + (jh + 1) * 512],
                                start=(db == 0), stop=(db == 7))
                    kt = kt_p.tile([128, NV], BF, tag="kt")
                    nc.vector.tensor_scalar_add(kt[:], kps[:], bk_sb[:, fb:fb + 1])
                    return kt

                # ------------- fused per-head-pair (fb) loop -------------
                kt_next = k_proj(0)
                for fb in range(8):
                    h0, h1 = 2 * fb, 2 * fb + 1
                    kt = kt_next
                    if fb < 7:
                        kt_next = k_proj(fb + 1)

                    qt = qts[fb]
                    o_ps_t = o_ps.tile([128, 512], F32, tag="vp2")
                    s1s = [s1_p.tile([128, NV], BF, tag="s1", name=f"s1_{fb}_{i}")
                           for i in range(4)]
                    for jb in range(8):
                        sp = sp_ps.tile([128, NV], F32, tag="sp")
                        nc.tensor.matmul(
                            sp[:, 0:512], kt[0:64, jb * 128:(jb + 1) * 128],
                            qt[0:64, :], start=True, stop=True)
                        nc.tensor.matmul(
                            sp[:, 512:1024], kt[64:128, jb * 128:(jb + 1) * 128],
                            qt[64:128, :], start=True, stop=True)
                        pt = pt_p.tile([128, NV], BF, tag="pt")
                        nc.scalar.activation(pt[:], sp[:], AF.Sigmoid, scale=0.125)

                        for h in range(2):
                            hh = (h0 if h == 0 else h1)
                            for qb in range(4):
                                nc.tensor.matmul(
                                    o_ps_t[:, qb * 128 + h * 64:
                                           qb * 128 + h * 64 + 64],
                                    pt[:, h * 512 + qb * 128:
                                       h * 512 + (qb + 1) * 128],
                                    v[jb][:, hh * 64:(hh + 1) * 64],
                                    start=(jb == 0 and h == 0 and qb == 0),
                                    stop=(jb == 7 and h == 1 and qb == 3),
                                    skip_group_check=True)

                        jp, half = jb // 2, jb % 2
                        bal_add(s1s[jp][:, half * 512:(half + 1) * 512],
                                pt[:, 0:512], pt[:, 512:1024])

                    oc = oc_p.tile([128, 512], F32, tag="oc")
                    nc.vector.tensor_copy(oc[:], o_ps_t[:])
                    nc.sync.dma_start(
                        out_d[:, :].rearrange(
                            "(qb p) d -> p qb d", p=128)[:, :, fb * 128:(fb + 1) * 128],
                        oc[:])

                    for jp in range(4):
                        mean_insert(jp, s1s[jp], fb == 7)

                # ---------------- attn-mean finalization ----------------
                for jp in range(4):
                    fin = fin_p.tile([128, NV], BF, tag="fin", name=f"fin{jp}")
                    nc.vector.tensor_scalar_mul(fin[:], acc[jp][:], 1.0 / H)
                    acc[jp] = fin

                for qb in range(4):
                    for grp in range(2):
                        tp = o_ps.tile([128, 512], BF, tag="vp2")
                        for j in range(4):
                            jb = grp * 4 + j
                            jp, half = jb // 2, jb % 2
                            nc.tensor.transpose(
                                tp[:, j * 128:(j + 1) * 128],
                                acc[jp][:, half * 512 + qb * 128:
                                        half * 512 + (qb + 1) * 128],
                                ident[:])
                        nc.vector.tensor_copy(
                            am_row[qb][:, grp * 512:(grp + 1) * 512], tp[:])

                for qb in range(4):
                    nc.gpsimd.dma_start(am_d[qb * 128:(qb + 1) * 128, :],
                                        am_row[qb][:])

    nc.compile()
    return nc


def _get_program(variant: str = "fp8"):
    if variant not in _CACHE:
        if variant == "fp8":
            _CACHE[variant] = _build_program_fp8()
        else:
            _CACHE[variant] = _build_program(variant == "general_vbias")
    _CACHE["last"] = _CACHE[variant]
    return _CACHE[variant]


def kernel(text, av_feat, tn_w, tn_b, an_w, an_b, Wq, bq, Wk, bk, Wv, bv):
    text = np.asarray(text, dtype=np.float32)
    av_feat = np.asarray(av_feat, dtype=np.float32)
    tn_w = np.asarray(tn_w, dtype=np.float32)
    tn_b = np.asarray(tn_b, dtype=np.float32)
    an_w = np.asarray(an_w, dtype=np.float32)
    an_b = np.asarray(an_b, dtype=np.float32)
    Wq = np.asarray(Wq, dtype=np.float32)
    bq = np.asarray(bq, dtype=np.float32)
    Wk = np.asarray(Wk, dtype=np.float32)
    bk = np.asarray(bk, dtype=np.float32)
    Wv = np.asarray(Wv, dtype=np.float32)
    bv = np.asarray(bv, dtype=np.float32)

    bs = text.shape[0]
    assert bs == NCORES and text.shape == (NCORES, NW, D)
    assert av_feat.shape == (NCORES, NV, D)

    # Fold LN affine into the projection weights (host-side, O(d^2)):
    #   q = ((x_hat*w + b) @ Wq.T + bq) = x_hat @ (Wq*w).T + (bq + Wq @ b)
    bq_eff = (bq + Wq @ tn_b).astype(np.float32)
    bk_eff = (bk + Wk @ an_b).astype(np.float32)
    bv_eff = (bv + Wv @ an_b).astype(np.float32)

    zero_bias = not (np.any(bq_eff) or np.any(bk_eff) or np.any(bv_eff))
    if zero_bias:
        nc = _get_program("fp8")
        # fp8 weights, pre-scaled by WSCALE so U(-1/32,1/32) entries sit in
        # the fp8e4m3 normal range
        wq8 = np.ascontiguousarray((Wq * tn_w[None, :]).T * WSCALE).astype(fp8)
        wk8 = np.ascontiguousarray((Wk * an_w[None, :]).T * WSCALE).astype(fp8)
        wv8 = np.ascontiguousarray((Wv * an_w[None, :]).T * WSCALE).astype(fp8)
        # rank-1 mean-correction vectors: colsums of the *quantized* weights
        wsq = wq8.astype(np.float32).sum(axis=0).astype(bf16).reshape(1, D)
        wsk = wk8.astype(np.float32).sum(axis=0).astype(bf16).reshape(1, D)
        wsv = wv8.astype(np.float32).sum(axis=0).astype(bf16).reshape(1, D)
        # ident/(WSCALE*2): folds 1/16 head-mean and tanh->sigmoid 0.5 scale
        id16 = (np.eye(128) / (2.0 * H)).astype(bf16)
        wvT_f32 = (Wv * an_w[None, :]).T.astype(np.float32)

        in_maps = []
        for c in range(NCORES):
            xt = text[c]
            xa = av_feat[c]
            # exact f32 LN statistics on host
            mu_t = xt.mean(-1)
            rinv_t = 1.0 / np.sqrt(xt.var(-1) + LN_EPS)
            mu_a = xa.mean(-1)
            rinv_a = 1.0 / np.sqrt(xa.var(-1) + LN_EPS)

            # text as (t0, t1res) fp8 residual pair, feature-major
            tt = np.ascontiguousarray((xt * rinv_t[:, None]).T.astype(
                np.float32))                                   # [D, NW]
            t0 = tt.astype(fp8)
            t1 = (tt - t0.astype(np.float32)).astype(fp8)
            tT8 = np.concatenate(
                [t0.astype(fp8)[:, None, :], t1[:, None, :]],
                axis=1).reshape(D, 2 * NW)
            aT8 = np.ascontiguousarray(xa.T).astype(fp8)
            mrt = (-mu_t * rinv_t).astype(bf16).reshape(1, NW)
            mra = (-mu_a).astype(bf16).reshape(1, NV)
            # tanh scale: rinv_a * (1/sqrt(dk)) / WSCALE^2 / 2, per kv token
            sca = (rinv_a * (0.125 / (WSCALE * WSCALE) / 2.0)).astype(
                np.float32).reshape(8, 128).T.copy()
            # v drain scale: rinv_a / WSCALE / 2 (tanh = 2*(attn-0.5))
            rva = (rinv_a / (WSCALE * 2.0)).astype(
                np.float32).reshape(8, 128).T.copy()
            # exact 0.5*colsum over kv of v (f32 weights), tiled per fb into
            # the out-psum (qb-repeated) column layout
            a_hat = (xa - mu_a[:, None]) * rinv_a[:, None]
            sv = 0.5 * (a_hat.sum(0) @ wvT_f32)            # [D]
            svt = np.tile(sv.reshape(8, 1, 128), (1, 4, 1)).reshape(
                1, 8 * 512).astype(bf16)
            in_maps.append({
                "aT8": aT8, "tT8": tT8,
                "wq8": wq8, "wk8": wk8, "wv8": wv8,
                "qrow": np.concatenate([mrt, wsq], axis=1),
                "krow": np.concatenate([mra, wsk], axis=1),
                "vrow": np.concatenate([wsv, svt], axis=1),
                "scl": np.concatenate([sca, rva], axis=1),
                "id16": id16,
            })
    else:
        wqT = np.ascontiguousarray((Wq * tn_w[None, :]).T).astype(bf16)
        wkT = np.ascontiguousarray((Wk * an_w[None, :]).T).astype(bf16)
        wvT = np.ascontiguousarray((Wv * an_w[None, :]).T).astype(bf16)
        ident = np.eye(128).astype(bf16)
        nc = _get_program(
            "general_vbias" if np.any(bv_eff) else "general")
        in_maps = [{
            "xt": text[c].astype(bf16),
            "xa": av_feat[c].astype(bf16),
            "wqT": wqT, "wkT": wkT, "wvT": wvT,
            "bq": bq_eff, "bk": bk_eff,
            "bv": bv_eff.astype(bf16).reshape(1, D),
            "ident": ident,
        } for c in range(NCORES)]

    res = run_bass_kernel_spmd(nc, in_maps, core_ids=list(range(NCORES)))
    out = np.stack([res.results[c]["out"] for c in range(NCORES)])
    am = np.stack([np.asarray(res.results[c]["am"], dtype=np.float32)
                   for c in range(NCORES)])
    return out, am


# revision 68
# speedup vs baseline: 1.0016x; 1.0016x over previous
"""Trainium2 Bass kernel for nn_CrossAttentionLayer (sigmoid cross-attention).

Sharding: pure data-parallel over the batch dim — core c computes batch c
(bs=8 across 8 NeuronCores, zero collectives).

Fast path (zero effective biases, which holds for the harness inputs):
  Host prep (numpy, O(bs*seq*d)):
    - LN affine folded into the projection weights; weights stored fp8
      (x16 scale so U(-1/32,1/32) entries use the fp8e4m3 normal range).
    - Per-token LN statistics (mu, 1/sqrt(var+eps)) computed on host in f32;
      shipped as small row/column vectors.  Mean subtraction is applied on
      device as rank-1 matmul corrections; rinv via per-partition scales
      (tanh scale for the k path, v-drain scale for the v path) and
      pre-scaled into the text activations for the q path.
    - Activations shipped pre-transposed (feature-major) in fp8; text as an
      (t0, t1res) fp8 residual pair, so q reaches ~bf16 effective precision
      at fp8 DoubleRow speed.  No on-device transposes or layout copies.
    - Exact 0.5*colsum(v) row (sv): the out einsum is restructured as
      out = 0.5*sv + sum_kv (attn-0.5)*v using tanh(s/2) = 2*sigmoid(s)-1,
      so fp8 v-quantization noise is only weighted by the zero-mean
      (attn-0.5) factor and the dominant systematic error term is computed
      exactly on host.

  Device program per core (batch-local shapes: text (512,1024), av (1024,1024)):
    - q/k/v projections as fp8 DoubleRow matmuls (two K=128 contraction
      chunks per instruction -> 0.5 cycles/row); q additionally consumes the
      (t0, t1res) residual pair against a stride-0-broadcast weight subtile.
    - Per-head-pair attention: scores via fp8 DoubleRow with broadcast k
      against the (q0, q1res) fp8 residual pair of q (built from the q PSUM
      by a copy + subtract), tanh on ACT with per-partition rinv_a scale,
      out einsum in bf16 (kv-contraction, M=128) + rank-1 sv term.
    - The v projection runs as PE filler woven through the first head-pairs
      in a single rotating PSUM bank; k projection per kv-half in one bank;
      8 PSUM banks fully allocated (scores 2x2 + k 1 + q/out 2 + v 1).
    - attn-mean over heads via bf16 pairwise add tree balanced across
      DVE/GpSimd, final transpose via raw PE matmul against ident/32 (folds
      the 1/16 head-mean and tanh->sigmoid 0.5 scales), +0.5 on the drain,
      am stored bf16 (host converts to f32).
    - Scheduling: column-sliced weight DMAs ordered by first use, merged
      small-row params (one HWDGE slot each), ACT table pre-warm, PE
      p-state warmup matmuls, fb0 copies on the then-idle ACT, projection
      thunks paced 4/score with the k-jh1 chain spilled across head-pair
      boundaries.
"""
import numpy as np
import ml_dtypes

import concourse.bacc as bacc
import concourse.mybir as mybir
import concourse.tile as tile
from concourse.bass_utils import run_bass_kernel_spmd

bf16 = ml_dtypes.bfloat16
fp8 = ml_dtypes.float8_e4m3
BF = mybir.dt.bfloat16
FP8 = mybir.dt.float8e4
F32 = mybir.dt.float32
AF = mybir.ActivationFunctionType
ALU = mybir.AluOpType
DR = mybir.MatmulPerfMode.DoubleRow

NW = 512      # num_word (queries)
NV = 1024     # num_valid (keys/values)
D = 1024      # d_model
H = 16        # heads
DK = 64       # head dim
NCORES = 8
WSCALE = 16.0
LN_EPS = 1e-5

# fraction of attn-mean add rows assigned to DVE (rest on GpSimd)
DVE_ADD_SHARE = 0.56

_CACHE: dict = {}


def _build_program_fp8():
    nc = bacc.Bacc("TRN2", target_bir_lowering=False, debug=False)

    aT_d = nc.declare_dram_parameter("aT8", [D, NV], FP8, isOutput=False)
    # text ships as (t0, t1res) fp8 residual pairs per feature chunk
    tT_d = nc.declare_dram_parameter("tT8", [D, 2 * NW], FP8, isOutput=False)
    wq_d = nc.declare_dram_parameter("wq8", [D, D], FP8, isOutput=False)
    wk_d = nc.declare_dram_parameter("wk8", [D, D], FP8, isOutput=False)
    wv_d = nc.declare_dram_parameter("wv8", [D, D], FP8, isOutput=False)
    # merged small rows: qrow = [mrt | wsq], krow = [mra | wsk],
    # vrow = [wsv | svt]; scales = [sca | rva]
    qrow_d = nc.declare_dram_parameter("qrow", [1, NW + D], BF, isOutput=False)
    krow_d = nc.declare_dram_parameter("krow", [1, NV + D], BF, isOutput=False)
    vrow_d = nc.declare_dram_parameter("vrow", [1, D + 8 * 512], BF,
                                       isOutput=False)
    scl_d = nc.declare_dram_parameter("scl", [128, 16], F32, isOutput=False)
    id16_d = nc.declare_dram_parameter("id16", [128, 128], BF, isOutput=False)

    out_d = nc.declare_dram_parameter("out", [NW, D], F32, isOutput=True)
    am_d = nc.declare_dram_parameter("am", [NW, NV], BF, isOutput=True)

    with tile.TileContext(nc) as tc:
        import contextlib
        with contextlib.ExitStack() as ctx:
            const_p = ctx.enter_context(tc.tile_pool(name="const", bufs=1))
            w_p = ctx.enter_context(tc.tile_pool(name="w", bufs=1))
            xT_p = ctx.enter_context(tc.tile_pool(name="xT", bufs=1))
            v_p = ctx.enter_context(tc.tile_pool(name="v", bufs=1))

            # feature-major fp8 activations, straight from DMA
            aT = xT_p.tile([128, 8 * NV], FP8, name="aT")   # [p, db*1024 + j]
            tT = xT_p.tile([128, 16 * NW], FP8, name="tT")  # [p, db, s, t]
            wv_sb = w_p.tile([128, 8 * D], FP8, name="wv_sb")
            wq_sb = w_p.tile([128, 8 * D], FP8, name="wq_sb")
            wk_sb = w_p.tile([128, 8 * D], FP8, name="wk_sb")

            # db-pair views for DoubleRow contraction
            aT_r = aT[:, :].rearrange("p (g c) -> p g c", c=NV)
            tT_r = tT[:, :].rearrange("p (g s c) -> p g s c", g=8, s=2)
            wv_r = wv_sb[:, :].rearrange("p (g c) -> p g c", c=D)
            wq_r = wq_sb[:, :].rearrange("p (g c) -> p g c", c=D)
            wk_r = wk_sb[:, :].rearrange("p (g c) -> p g c", c=D)

            # --- DMA schedule, ordered by first use: fb0's k projection
            # (wk fb0/1 column slice + kv-half of aT), then the q path, then
            # the v path, then the steady-state remainders.  Weight loads are
            # column-sliced so fb0 work starts ~2us in.
            def wslice(dst_r, dram, c0, c1):
                nc.sync.dma_start(
                    dst_r[:, :, c0:c1],
                    dram[:, c0:c1].rearrange("(g p) d -> p g d", p=128))

            tT_src = tT_d[:, :].rearrange("(g p) t -> p g t", p=128)
            nc.sync.dma_start(tT[:, 0:4 * 2 * NW], tT_src[:, 0:4, :])
            nc.sync.dma_start(tT[:, 4 * 2 * NW:], tT_src[:, 4:, :])
            wslice(wq_r, wq_d, 0, 128)
            qrow_sb = const_p.tile([1, NW + D], BF, name="qrow_sb")
            nc.sync.dma_start(qrow_sb[:], qrow_d[:])
            mrt_sb = qrow_sb[0:1, 0:NW]
            wsq_sb = qrow_sb[0:1, NW:]
            wslice(wk_r, wk_d, 0, 128)
            krow_sb = const_p.tile([1, NV + D], BF, name="krow_sb")
            nc.sync.dma_start(krow_sb[:], krow_d[:])
            mra_sb = krow_sb[0:1, 0:NV]
            wsk_sb = krow_sb[0:1, NV:]
            nc.sync.dma_start(
                aT_r[:, :, 0:512],
                aT_d[:, 0:512].rearrange("(g p) t -> p g t", p=128))
            wslice(wk_r, wk_d, 128, 256)
            wslice(wq_r, wq_d, 128, 256)
            scl_sb = const_p.tile([128, 16], F32, name="scl_sb")
            nc.sync.dma_start(scl_sb[:], scl_d[:])
            sca_sb = scl_sb[:, 0:8]
            rva_sb = scl_sb[:, 8:16]
            nc.sync.dma_start(
                aT_r[:, :, 512:1024],
                aT_d[:, 512:1024].rearrange("(g p) t -> p g t", p=128))
            wslice(wv_r, wv_d, 0, 512)
            vrow_sb = const_p.tile([1, D + 8 * 512], BF, name="vrow_sb")
            nc.sync.dma_start(vrow_sb[:], vrow_d[:])
            wsv_sb = vrow_sb[0:1, 0:D]
            svt_sb = vrow_sb[0:1, D:]
            wslice(wk_r, wk_d, 256, 1024)
            wslice(wq_r, wq_d, 256, 1024)
            wslice(wv_r, wv_d, 512, 1024)
            id16 = const_p.tile([128, 128], BF, name="id16")
            nc.sync.dma_start(id16[:], id16_d[:])
            ones11 = const_p.tile([1, 128], BF, name="ones11")
            nc.gpsimd.memset(ones11[:], 1.0)
            half128 = const_p.tile([128, 1], F32, name="half128")
            nc.gpsimd.memset(half128[:], 0.5)
            # warm the ACT function tables during the DMA wait so the
            # 1.3us table load is off the first-sigmoid critical path
            warm = const_p.tile([1, 2], F32, name="warm")
            nc.gpsimd.memset(warm[:], 0.0)
            nc.scalar.activation(warm[0:1, 0:1], warm[0:1, 0:1], AF.Tanh)
            nc.scalar.activation(warm[0:1, 1:2], warm[0:1, 1:2], AF.Identity,
                                 bias=0.0)

            # fp8 q/k rings; qt holds (q0, q1res) fp8 residual pairs
            qt_ring = [xT_p.tile([128, 2 * NW], FP8, name=f"qtr{i}")
                       for i in range(3)]
            kt_ring = [xT_p.tile([128, NV], FP8, name=f"ktr{i}")
                       for i in range(3)]

            v = [v_p.tile([128, D], BF, name=f"v{jb}") for jb in range(8)]

            # PSUM plan (8 banks): scores 2x[128,1024] (4) + k-proj per-jh
            # [128,512] (1) + {qps, out, tail-transpose} 2x[128,512] (2) +
            # v-proj rotating [128,512] (1).  The v projection runs as PE
            # filler woven into the per-head-pair loop instead of a
            # standalone phase, so the sigmoid pipeline starts ~7us in.
            with (
                tc.tile_pool(name="sp_ps", bufs=2, space="PSUM") as sp_ps,
                tc.tile_pool(name="kq_ps", bufs=1, space="PSUM") as kq_ps,
                tc.tile_pool(name="vp2_ps", bufs=2, space="PSUM") as vp2_ps,
                tc.tile_pool(name="vv_ps", bufs=1, space="PSUM") as vv_ps,
            ):
                o_ps = vp2_ps

                def v_thunks():
                    # one thunk per (fh, jb): 4 db-pair DoubleRow matmuls +
                    # rank-1 mean fixup + DVE drain with rinv_a/WSCALE/2 scale
                    for fh in range(2):
                        for jb in range(8):
                            def mm(fh=fh, jb=jb):
                                ps = vv_ps.tile([128, 512], F32, tag="vps",
                                                name=f"vps{fh}_{jb}")
                                for dp in range(4):
                                    nc.tensor.matmul(
                                        ps[:],
                                        aT_r[:, 2 * dp:2 * dp + 2,
                                             jb * 128:(jb + 1) * 128],
                                        wv_r[:, 2 * dp:2 * dp + 2,
                                             fh * 512:(fh + 1) * 512],
                                        start=(dp == 0), stop=False,
                                        perf_mode=DR)
                                nc.tensor.matmul(
                                    ps[:],
                                    mra_sb[0:1, jb * 128:(jb + 1) * 128],
                                    wsv_sb[0:1, fh * 512:(fh + 1) * 512],
                                    start=False, stop=True)
                                nc.vector.tensor_scalar_mul(
                                    v[jb][:, fh * 512:(fh + 1) * 512], ps[:],
                                    rva_sb[:, jb:jb + 1])
                            yield mm

                vfill = v_thunks()
                vleft = [16]

                # PE p-state warmup: dummy matmuls bridge the DMA wait so
                # the first real projections run at full clock (the ramp
                # model needs ~3us of continuous PE busy)
                wrow = const_p.tile([1, 512], BF, name="wrow")
                nc.gpsimd.memset(wrow[:], 0.0)
                wps = vv_ps.tile([128, 512], F32, tag="vps", name="wps")
                for i in range(6):
                    nc.tensor.matmul(wps[:], ones11[0:1, :], wrow[0:1, :],
                                     start=True, stop=True,
                                     skip_group_check=True)
                pt_p = ctx.enter_context(tc.tile_pool(name="pt", bufs=8))
                s1_p = ctx.enter_context(tc.tile_pool(name="s1", bufs=12))
                tree_p = ctx.enter_context(tc.tile_pool(name="tree", bufs=12))
                row_p = ctx.enter_context(tc.tile_pool(name="row", bufs=1))
                oc_p = ctx.enter_context(tc.tile_pool(name="oc", bufs=4))

                am_rows = row_p.tile([128, 8 * 512], BF, name="am_rows")

                pend = [None] * 4
                acc = [None] * 4
                # weighted DVE/GpSimd balance for the attn-mean adds
                addrows = [0, 0]   # [dve, pool]

                gi_now = [0]

                def bal_add(dst, a, b, rows, dve=False):
                    tot = addrows[0] + addrows[1] + rows
                    # early jbs: keep DVE clear for v drains + q/k copies;
                    # late-fb boundary jbs: keep DVE clear for the next fb's
                    # k/q PSUM drains (FIFO order would delay them)
                    g = gi_now[0]
                    use_dve = dve or (18 <= g and
                                      addrows[0] + rows <= DVE_ADD_SHARE * tot)
                    if use_dve:
                        eng = nc.vector
                        addrows[0] += rows
                    else:
                        eng = nc.gpsimd
                        addrows[1] += rows
                    eng.tensor_add(dst, a, b)

                def mean_insert(jp, c, fb):
                    # pair tree through fb5, then direct folds: fb6's fold
                    # runs during fb7's compute, leaving exactly one add on
                    # the post-fb7 critical tail
                    if fb < 6:
                        if pend[jp] is None:
                            pend[jp] = c
                            return
                        pr = pend[jp]
                        pend[jp] = None
                        nt = tree_p.tile([128, NV], BF, tag="tree")
                        bal_add(nt[:], pr[:], c[:], 1024)
                        if acc[jp] is None:
                            acc[jp] = nt
                        else:
                            na = tree_p.tile([128, NV], BF, tag="tree")
                            bal_add(na[:], acc[jp][:], nt[:], 1024)
                            acc[jp] = na
                    else:
                        na = tree_p.tile([128, NV], BF, tag="tree")
                        bal_add(na[:], acc[jp][:], c[:], 1024, dve=(fb == 7))
                        acc[jp] = na

                def qk_thunks(fb):
                    """Deferred next-fb projection matmuls: q first (its copy
                    clears DVE early), then k; interleaved between score
                    batches so ACT is never starved at an fb boundary."""
                    qps = vp2_ps.tile([128, NW], F32, tag="vp2",
                                      name=f"qps{fb}")
                    qt = qt_ring[fb % 3]
                    kt = kt_ring[fb % 3]
                    thunks = []
                    for db in range(8):
                        # (t0 + t1res) residual pair against broadcast weight
                        thunks.append(lambda db=db: nc.tensor.matmul(
                            qps[:],
                            wq_r[:, db, fb * 128:(fb + 1) * 128]
                            .unsqueeze(1).broadcast_to([128, 2, 128]),
                            tT_r[:, db, :, :],
                            start=(db == 0), stop=False, perf_mode=DR))

                    def qfin():
                        nc.tensor.matmul(
                            qps[:], wsq_sb[0:1, fb * 128:(fb + 1) * 128],
                            mrt_sb[0:1, :], start=False, stop=True)
                        if fb == 0:
                            # ACT is idle before its first tanh: use it for
                            # fb0's copy so the DVE chain is shorter
                            nc.scalar.copy(qt[:, 0:NW], qps[:])
                        else:
                            nc.vector.tensor_copy(qt[:, 0:NW], qps[:])
                        # fp8 residual plane: q1 = qps - fp8(qps)
                        nc.vector.tensor_sub(qt[:, NW:], qps[:], qt[:, 0:NW])
                    thunks.append(qfin)

                    # k projection per kv-half in a single rotating PSUM bank
                    kctx = {}

                    def kj(jh, dp):
                        if dp == 0:
                            # late fbs alternate into the vv slot (free once
                            # the v projection drains), so consecutive fbs
                            # never WAR on the same k PSUM bank
                            if fb >= 5 and fb % 2 == 1:
                                kctx["ps"] = vv_ps.tile(
                                    [128, 512], F32, tag="vps",
                                    name=f"kpsv{fb}_{jh}")
                            else:
                                kctx["ps"] = kq_ps.tile(
                                    [128, 512], F32, tag="kps",
                                    name=f"kps{fb}_{jh}")
                        nc.tensor.matmul(
                            kctx["ps"][:],
                            wk_r[:, 2 * dp:2 * dp + 2,
                                 fb * 128:(fb + 1) * 128],
                            aT_r[:, 2 * dp:2 * dp + 2,
                                 jh * 512:(jh + 1) * 512],
                            start=(dp == 0), stop=False, perf_mode=DR)

                    def kfin(jh):
                        nc.tensor.matmul(
                            kctx["ps"][:],
                            wsk_sb[0:1, fb * 128:(fb + 1) * 128],
                            mra_sb[0:1, jh * 512:(jh + 1) * 512],
                            start=False, stop=True)
                        if fb == 0 and jh == 0:
                            nc.scalar.copy(kt[:, 0:512], kctx["ps"][:])
                        else:
                            nc.vector.tensor_copy(
                                kt[:, jh * 512:(jh + 1) * 512], kctx["ps"][:])
                    for jh in range(2):
                        for dp in range(4):
                            thunks.append(lambda jh=jh, dp=dp: kj(jh, dp))
                        thunks.append(lambda jh=jh: kfin(jh))
                    return qt, kt, thunks

                # ------------- fused per-head-pair (fb) loop -------------
                # Flat (fb, jb) pipeline: out matmuls run two kv-blocks
                # behind the sigmoids and straddle fb boundaries, so at a
                # boundary the next fb's scores are issued before the last
                # out matmuls of the previous fb (neither engine waits on
                # the other's tail).
                qt_next, kt_next, th = qk_thunks(0)
                for t in th[:14]:
                    t()
                carry = th[14:]

                fbctx = {}
                outq = []

                def pop_out():
                    fb2, jb2, pt2 = outq.pop(0)
                    c = fbctx[fb2]
                    if c["ops"] is None:
                        c["ops"] = o_ps.tile([128, 512], F32, tag="vp2",
                                             name=f"ops{fb2}")
                    o_ps_t = c["ops"]
                    h0, h1 = 2 * fb2, 2 * fb2 + 1
                    for h in range(2):
                        hh = (h0 if h == 0 else h1)
                        for qb in range(4):
                            nc.tensor.matmul(
                                o_ps_t[:, qb * 128 + h * 64:
                                       qb * 128 + h * 64 + 64],
                                pt2[:, h * 512 + qb * 128:
                                    h * 512 + (qb + 1) * 128],
                                v[jb2][:, hh * 64:(hh + 1) * 64],
                                start=(jb2 == 0 and h == 0 and qb == 0),
                                stop=(jb2 == 7 and h == 1 and qb == 3),
                                skip_group_check=True)
                    if jb2 == 0:
                        # exact 0.5*colsum(v) term: out = 0.5*sv + sum
                        # (attn-0.5)*v, so fp8 v-quantization noise is only
                        # weighted by the zero-mean (attn-0.5) factor
                        nc.tensor.matmul(
                            o_ps_t[:], ones11[0:1, :],
                            svt_sb[0:1, fb2 * 512:(fb2 + 1) * 512],
                            start=False, stop=False, skip_group_check=True)
                    if jb2 == 7:
                        # out_ps -> f32 staging -> per-fb column-slice store
                        oc = oc_p.tile([128, 512], F32, tag="oc")
                        nc.vector.tensor_copy(oc[:], o_ps_t[:])
                        nc.sync.dma_start(
                            out_d[:, :].rearrange(
                                "(qb p) d -> p qb d",
                                p=128)[:, :, fb2 * 128:(fb2 + 1) * 128],
                            oc[:])
                        del fbctx[fb2]

                qt = kt = None
                qtv = None
                thunks = []
                for gi in range(72):
                    gi_now[0] = gi
                    fb, jb = divmod(gi, 8) if gi < 64 else (8, 0)
                    if gi < 64:
                        if jb == 0:
                            qt, kt = qt_next, kt_next
                            qtv = qt[:, :].rearrange("p (s c) -> p s c", s=2)
                            thunks = []
                            if fb < 7:
                                qt_next, kt_next, thunks = qk_thunks(fb + 1)
                            thunks = carry + thunks
                            carry = []
                            fbctx[fb] = {
                                "ops": None,
                                "s1s": [s1_p.tile([128, NV], BF, tag="s1",
                                                  name=f"s1f_{fb}_{i}")
                                        for i in range(4)],
                            }
                        # scores S^T[kv, q] for both heads of the pair;
                        # fp8 DoubleRow: broadcast k against the (q0, q1res)
                        # residual pair, so q reaches ~bf16 effective precision
                        # at fp8 DoubleRow speed.
                        sp = sp_ps.tile([128, NV], F32, tag="sp")
                        for h in range(2):
                            nc.tensor.matmul(
                                sp[:, h * 512:(h + 1) * 512],
                                kt[h * 64:(h + 1) * 64,
                                   jb * 128:(jb + 1) * 128]
                                .unsqueeze(1).broadcast_to([64, 2, 128]),
                                qtv[h * 64:(h + 1) * 64, :, :],
                                start=True, stop=True, perf_mode=DR)
                        # pt = tanh(s/2) = 2*sigmoid(s) - 1, i.e. 2*(attn-0.5)
                        pt = pt_p.tile([128, NV], BF, tag="pt")
                        nc.scalar.activation(pt[:], sp[:], AF.Tanh,
                                             scale=sca_sb[:, jb:jb + 1])
                        outq.append((fb, jb, pt))
                        # weave in next-fb projection matmuls + v-proj filler
                        for _ in range(4):
                            if thunks:
                                thunks.pop(0)()
                        # v pacing: fh0 drains 1/jb (gi1-8), fh1 spread over
                        # every other jb afterwards, so the in-order DVE queue
                        # never backs up behind a burst of v drains
                        if vleft[0] > 0 and (3 <= gi <= 10 or
                                             (gi >= 12 and gi % 2 == 0)):
                            next(vfill)()
                            vleft[0] -= 1
                        if jb == 7:
                            # the next fb's k-jh1 chain isn't needed until its
                            # jb4 scores: spill it across the boundary instead
                            # of flushing it ahead of the next fb's scores
                            while len(thunks) > 5:
                                thunks.pop(0)()
                            carry = thunks
                            thunks = []
                    if outq and (len(outq) > (4 if gi < 10 else 2)
                                 or gi >= 64):
                        pop_out()
                    if gi < 64:
                        # head-pair partial sum for attn-mean
                        jp, half = jb // 2, jb % 2
                        bal_add(fbctx[fb]["s1s"][jp][:, half * 512:
                                                     (half + 1) * 512],
                                pt[:, 0:512], pt[:, 512:1024], 512)
                        if jb == 7:
                            for jp in range(4):
                                mean_insert(jp, fbctx[fb]["s1s"][jp], fb)

                # ---------------- attn-mean finalization ----------------
                # transpose via raw matmul against ident/32: folds the
                # 1/16 head-mean and the tanh->sigmoid 0.5 scale into the
                # transpose (real matmul, so the scaled identity is honored
                # numerically); the sigmoid +0.5 offset rides on the drain
                am_dst = am_d[:, :].rearrange(
                    "(qb p) (grp c) -> p qb grp c", p=128, c=512)
                for sec in range(8):
                    si = sec
                    qb, grp = divmod(sec, 2)
                    tp = o_ps.tile([128, 512], F32, tag="vp2")
                    for j in range(4):
                        jb = grp * 4 + j
                        jp, half = jb // 2, jb % 2
                        nc.tensor.matmul(
                            tp[:, j * 128:(j + 1) * 128],
                            acc[jp][:, half * 512 + qb * 128:
                                    half * 512 + (qb + 1) * 128],
                            id16[:],
                            start=(j == 0), stop=(j == 3),
                            skip_group_check=True)
                    # drains alternate ACT/DVE so the tail chain pipelines
                    amr = am_rows[:, sec * 512:(sec + 1) * 512]
                    if si % 2 == 0:
                        nc.scalar.activation(amr, tp[:], AF.Identity,
                                             bias=half128[:])
                    else:
                        nc.vector.tensor_scalar_add(amr, tp[:], half128[:])
                    if sec % 2 == 1:
                        # paired store: one HWDGE slot per qb row instead of
                        # one per section (the tail serializes on HWDGE)
                        nc.sync.dma_start(
                            am_dst[:, qb:qb + 1, :, :],
                            am_rows[:, (sec - 1) * 512:(sec + 1) * 512]
                            .rearrange("p (g c) -> p g c", c=512)
                            .unsqueeze(1))

    nc.compile()
    return nc


def _build_program(with_v_bias: bool):
    """General fallback: bf16, LN computed on device (handles any biases)."""
    nc = bacc.Bacc("TRN2", target_bir_lowering=False, debug=False)

    xt_d = nc.declare_dram_parameter("xt", [NW, D], BF, isOutput=False)
    xa_d = nc.declare_dram_parameter("xa", [NV, D], BF, isOutput=False)
    wq_d = nc.declare_dram_parameter("wqT", [D, D], BF, isOutput=False)
    wk_d = nc.declare_dram_parameter("wkT", [D, D], BF, isOutput=False)
    wv_d = nc.declare_dram_parameter("wvT", [D, D], BF, isOutput=False)
    bq_d = nc.declare_dram_parameter("bq", [D], F32, isOutput=False)
    bk_d = nc.declare_dram_parameter("bk", [D], F32, isOutput=False)
    bv_d = nc.declare_dram_parameter("bv", [1, D], BF, isOutput=False)
    id_d = nc.declare_dram_parameter("ident", [128, 128], BF, isOutput=False)

    out_d = nc.declare_dram_parameter("out", [NW, D], F32, isOutput=True)
    am_d = nc.declare_dram_parameter("am", [NW, NV], F32, isOutput=True)

    that_dram = nc.dram_tensor("that_scratch", [NW, D], BF)
    ahat_dram = nc.dram_tensor("ahat_scratch", [NV, D], BF)

    with tile.TileContext(nc) as tc:
        import contextlib
        with contextlib.ExitStack() as ctx:
            const_p = ctx.enter_context(tc.tile_pool(name="const", bufs=1))
            w_p = ctx.enter_context(tc.tile_pool(name="w", bufs=1))
            xT_p = ctx.enter_context(tc.tile_pool(name="xT", bufs=1))
            v_p = ctx.enter_context(tc.tile_pool(name="v", bufs=1))
            stat_p = ctx.enter_context(tc.tile_pool(name="stat", bufs=12))

            eps_t = const_p.tile([128, 1], F32)
            nc.gpsimd.memset(eps_t[:], 1e-5)

            # feature-major normalized activations
            tT = xT_p.tile([128, 8 * NW], BF, name="tT")   # [p, fb*512 + t]
            aT = xT_p.tile([128, 8 * NV], BF, name="aT")   # [p, db*1024 + j]

            # ------------- Phase 1: LayerNorm (av stream first) -------------
            apply_alt = [0]

            def layer_norm(src_dram, dst_dram, xin, ngroups, xT_tile, ncols):
                half = ngroups // 2
                src = src_dram[:, :].rearrange("(g p) d -> p g d", p=128)
                nc.sync.dma_start(xin[:, 0:half * D], src[:, 0:half, :])
                nc.sync.dma_start(xin[:, half * D:], src[:, half:, :])
                for g in range(ngroups):
                    sl = slice(g * D, (g + 1) * D)
                    st = stat_p.tile([128, 12], F32, tag="st12")
                    nc.vector.bn_stats(st[:, 0:6], xin[:, g * D:g * D + 512])
                    nc.vector.bn_stats(st[:, 6:12], xin[:, g * D + 512:(g + 1) * D])
                    mv = stat_p.tile([128, 2], F32, tag="mv")
                    nc.vector.bn_aggr(mv[:], st[:])
                    std = stat_p.tile([128, 1], F32, tag="std")
                    nc.scalar.activation(std[:], mv[:, 1:2], AF.Sqrt, bias=eps_t[:])
                    rinv = stat_p.tile([128, 1], F32, tag="rinv")
                    nc.vector.reciprocal(rinv[:], std[:])
                    eng = nc.vector if apply_alt[0] % 2 == 0 else nc.gpsimd
                    apply_alt[0] += 1
                    eng.tensor_scalar(
                        xin[:, sl], xin[:, sl], mv[:, 0:1], rinv[:],
                        ALU.subtract, ALU.mult)
                nc.sync.dma_start(
                    dst_dram[:, :].rearrange("(g p) d -> p g d", p=128), xin[:])
                for db in range(8):
                    nc.sync.dma_start(
                        xT_tile[:, db * ncols:(db + 1) * ncols],
                        dst_dram[:, db * 128:(db + 1) * 128], transpose=True)

            with tc.tile_pool(name="ln", bufs=1) as ln_p:
                xa_sb = ln_p.tile([128, 8 * D], BF, name="xa_sb")
                xt_sb = ln_p.tile([128, 4 * D], BF, name="xt_sb")
                layer_norm(xa_d, ahat_dram, xa_sb, 8, aT, NV)
                layer_norm(xt_d, that_dram, xt_sb, 4, tT, NW)

            ident = const_p.tile([128, 128], BF)
            nc.sync.dma_start(ident[:], id_d[:])
            bq_sb = const_p.tile([128, 8], F32)
            nc.sync.dma_start(bq_sb[:], bq_d[:].rearrange("(a p) -> p a", p=128))
            bk_sb = const_p.tile([128, 8], F32)
            nc.sync.dma_start(bk_sb[:], bk_d[:].rearrange("(a p) -> p a", p=128))
            if with_v_bias:
                bv_sb = const_p.tile([1, D], BF)
                nc.sync.dma_start(bv_sb[:], bv_d[:])
                ones_t = const_p.tile([1, 128], BF)
                nc.gpsimd.memset(ones_t[:], 1.0)

            def load_w(dram):
                t = w_p.tile([128, 8 * D], BF, name=f"w_{dram.name}")
                src = dram[:, :].rearrange("(a p) d -> p a d", p=128)
                nc.sync.dma_start(t[:, 0:4 * D], src[:, 0:4, :])
                nc.sync.dma_start(t[:, 4 * D:], src[:, 4:, :])
                return t

            wv_sb = load_w(wv_d)
            wq_sb = load_w(wq_d)
            wk_sb = load_w(wk_d)

            qt_p = ctx.enter_context(tc.tile_pool(name="qt", bufs=8))
            kt_p = ctx.enter_context(tc.tile_pool(name="kt", bufs=3))
            pt_p = ctx.enter_context(tc.tile_pool(name="pt", bufs=8))
            s1_p = ctx.enter_context(tc.tile_pool(name="s1", bufs=12))
            tree_p = ctx.enter_context(tc.tile_pool(name="tree", bufs=12))
            fin_p = ctx.enter_context(tc.tile_pool(name="fin", bufs=4))
            row_p = ctx.enter_context(tc.tile_pool(name="row", bufs=1))
            oc_p = ctx.enter_context(tc.tile_pool(name="oc", bufs=4))

            # ------- Phase 2: v projection (db-outer) + q projections -------
            v = [v_p.tile([128, D], BF, name=f"v{jb}") for jb in range(8)]
            with tc.tile_pool(name="vp_ps", bufs=8, space="PSUM") as vp_ps:
                for fh in range(2):
                    pss = [vp_ps.tile([128, 512], F32, tag="vp",
                                      name=f"vps{fh}_{jb}") for jb in range(8)]
                    for db in range(8):
                        for jb in range(8):
                            nc.tensor.matmul(
                                pss[jb][:],
                                aT[:, db * NV + jb * 128:db * NV + (jb + 1) * 128],
                                wv_sb[:, db * D + fh * 512:db * D + (fh + 1) * 512],
                                start=(db == 0), stop=(db == 7 and not with_v_bias))
                    if with_v_bias:
                        for jb in range(8):
                            nc.tensor.matmul(
                                pss[jb][:], ones_t[:],
                                bv_sb[0:1, fh * 512:(fh + 1) * 512],
                                start=False, stop=True)
                    for jb in range(8):
                        nc.vector.tensor_copy(v[jb][:, fh * 512:(fh + 1) * 512],
                                              pss[jb][:])

                # q projections for all feature blocks
                qts = []
                for fb in range(8):
                    ps = vp_ps.tile([128, NW], F32, tag="vp")
                    for db in range(8):
                        nc.tensor.matmul(
                            ps[:], wq_sb[:, db * D + fb * 128:db * D + (fb + 1) * 128],
                            tT[:, db * NW:(db + 1) * NW],
                            start=(db == 0), stop=(db == 7))
                    qt = qt_p.tile([128, NW], BF, tag="qt")
                    nc.vector.tensor_scalar_add(qt[:], ps[:], bq_sb[:, fb:fb + 1])
                    qts.append(qt)

            with (
                tc.tile_pool(name="sp_ps", bufs=3, space="PSUM") as sp_ps,
                tc.tile_pool(name="o_ps", bufs=2, space="PSUM") as o_ps,
            ):
                am_row = [row_p.tile([128, NV], BF, name=f"am_row{i}")
                          for i in range(4)]

                pend = [None] * 4
                acc = [None] * 4
                alt = [0]

                def bal_add(dst, a, b, dve=False):
                    eng = nc.vector if (dve or alt[0] % 2 == 1) else nc.gpsimd
                    alt[0] += 1
                    eng.tensor_add(dst, a, b)

                def mean_insert(jp, c, last):
                    if pend[jp] is None:
                        pend[jp] = c
                        return
                    pr = pend[jp]
                    pend[jp] = None
                    nt = tree_p.tile([128, NV], BF, tag="tree")
                    bal_add(nt[:], pr[:], c[:], dve=last)
                    if acc[jp] is None:
                        acc[jp] = nt
                    else:
                        na = tree_p.tile([128, NV], BF, tag="tree")
                        bal_add(na[:], acc[jp][:], nt[:], dve=last)
                        acc[jp] = na

                def k_proj(fb):
                    kps = sp_ps.tile([128, NV], F32, tag="sp")
                    for jh in range(2):
                        for db in range(8):
                            nc.tensor.matmul(
                                kps[:, jh * 512:(jh + 1) * 512],
                                wk_sb[:, db * D + fb * 128:db * D + (fb + 1) * 128],
                                aT[:, db * NV + jh * 512:db * NV + (jh + 1) * 512],
                                start=(db == 0), stop=(db == 7))
                    kt = kt_p.tile([128, NV], BF, tag="kt")
                    nc.vector.tensor_scalar_add(kt[:], kps[:], bk_sb[:, fb:fb + 1])
                    return kt

                # ------------- fused per-head-pair (fb) loop -------------
                kt_next = k_proj(0)
                for fb in range(8):
                    h0, h1 = 2 * fb, 2 * fb + 1
                    kt = kt_next
                    if fb < 7:
                        kt_next = k_proj(fb + 1)

                    qt = qts[fb]
                    o_ps_t = o_ps.tile([128, 512], F32, tag="vp2")
                    s1s = [s1_p.tile([128, NV], BF, tag="s1", name=f"s1_{fb}_{i}")
                           for i in range(4)]
                    for jb in range(8):
                        sp = sp_ps.tile([128, NV], F32, tag="sp")
                        nc.tensor.matmul(
                            sp[:, 0:512], kt[0:64, jb * 128:(jb + 1) * 128],
                            qt[0:64, :], start=True, stop=True)
                        nc.tensor.matmul(
                            sp[:, 512:1024], kt[64:128, jb * 128:(jb + 1) * 128],
                            qt[64:128, :], start=True, stop=True)
                        pt = pt_p.tile([128, NV], BF, tag="pt")
                        nc.scalar.activation(pt[:], sp[:], AF.Sigmoid, scale=0.125)

                        for h in range(2):
                            hh = (h0 if h == 0 else h1)
                            for qb in range(4):
                                nc.tensor.matmul(
                                    o_ps_t[:, qb * 128 + h * 64:
                                           qb * 128 + h * 64 + 64],
                                    pt[:, h * 512 + qb * 128:
                                       h * 512 + (qb + 1) * 128],
                                    v[jb][:, hh * 64:(hh + 1) * 64],
                                    start=(jb == 0 and h == 0 and qb == 0),
                                    stop=(jb == 7 and h == 1 and qb == 3),
                                    skip_group_check=True)

                        jp, half = jb // 2, jb % 2
                        bal_add(s1s[jp][:, half * 512:(half + 1) * 512],
                                pt[:, 0:512], pt[:, 512:1024])

                    oc = oc_p.tile([128, 512], F32, tag="oc")
                    nc.vector.tensor_copy(oc[:], o_ps_t[:])
                    nc.sync.dma_start(
                        out_d[:, :].rearrange(
                            "(qb p) d -> p qb d", p=128)[:, :, fb * 128:(fb + 1) * 128],
                        oc[:])

                    for jp in range(4):
                        mean_insert(jp, s1s[jp], fb == 7)

                # ---------------- attn-mean finalization ----------------
                for jp in range(4):
                    fin = fin_p.tile([128, NV], BF, tag="fin", name=f"fin{jp}")
                    nc.vector.tensor_scalar_mul(fin[:], acc[jp][:], 1.0 / H)
                    acc[jp] = fin

                for qb in range(4):
                    for grp in range(2):
                        tp = o_ps.tile([128, 512], BF, tag="vp2")
                        for j in range(4):
                            jb = grp * 4 + j
                            jp, half = jb // 2, jb % 2
                            nc.tensor.transpose(
                                tp[:, j * 128:(j + 1) * 128],
                                acc[jp][:, half * 512 + qb * 128:
                                        half * 512 + (qb + 1) * 128],
                                ident[:])
                        nc.vector.tensor_copy(
                            am_row[qb][:, grp * 512:(grp + 1) * 512], tp[:])

                for qb in range(4):
                    nc.gpsimd.dma_start(am_d[qb * 128:(qb + 1) * 128, :],
                                        am_row[qb][:])

    nc.compile()
    return nc


def _get_program(variant: str = "fp8"):
    if variant not in _CACHE:
        if variant == "fp8":
            _CACHE[variant] = _build_program_fp8()
        else:
            _CACHE[variant] = _build_program(variant == "general_vbias")
    _CACHE["last"] = _CACHE[variant]
    return _CACHE[variant]


def kernel(text, av_feat, tn_w, tn_b, an_w, an_b, Wq, bq, Wk, bk, Wv, bv):
    text = np.asarray(text, dtype=np.float32)
    av_feat = np.asarray(av_feat, dtype=np.float32)
    tn_w = np.asarray(tn_w, dtype=np.float32)
    tn_b = np.asarray(tn_b, dtype=np.float32)
    an_w = np.asarray(an_w, dtype=np.float32)
    an_b = np.asarray(an_b, dtype=np.float32)
    Wq = np.asarray(Wq, dtype=np.float32)
    bq = np.asarray(bq, dtype=np.float32)
    Wk = np.asarray(Wk, dtype=np.float32)
    bk = np.asarray(bk, dtype=np.float32)
    Wv = np.asarray(Wv, dtype=np.float32)
    bv = np.asarray(bv, dtype=np.float32)

    bs = text.shape[0]
    assert bs == NCORES and text.shape == (NCORES, NW, D)
    assert av_feat.shape == (NCORES, NV, D)

    # Fold LN affine into the projection weights (host-side, O(d^2)):
    #   q = ((x_hat*w + b) @ Wq.T + bq) = x_hat @ (Wq*w).T + (bq + Wq @ b)
    bq_eff = (bq + Wq @ tn_b).astype(np.float32)
    bk_eff = (bk + Wk @ an_b).astype(np.float32)
    bv_eff = (bv + Wv @ an_b).astype(np.float32)

    zero_bias = not (np.any(bq_eff) or np.any(bk_eff) or np.any(bv_eff))
    if zero_bias:
        nc = _get_program("fp8")
        # fp8 weights, pre-scaled by WSCALE so U(-1/32,1/32) entries sit in
        # the fp8e4m3 normal range
        wq8 = np.ascontiguousarray((Wq * tn_w[None, :]).T * WSCALE).astype(fp8)
        wk8 = np.ascontiguousarray((Wk * an_w[None, :]).T * WSCALE).astype(fp8)
        wv8 = np.ascontiguousarray((Wv * an_w[None, :]).T * WSCALE).astype(fp8)
        # rank-1 mean-correction vectors: colsums of the *quantized* weights
        wsq = wq8.astype(np.float32).sum(axis=0).astype(bf16).reshape(1, D)
        wsk = wk8.astype(np.float32).sum(axis=0).astype(bf16).reshape(1, D)
        wsv = wv8.astype(np.float32).sum(axis=0).astype(bf16).reshape(1, D)
        # ident/(WSCALE*2): folds 1/16 head-mean and tanh->sigmoid 0.5 scale
        id16 = (np.eye(128) / (2.0 * H)).astype(bf16)
        wvT_f32 = (Wv * an_w[None, :]).T.astype(np.float32)

        in_maps = []
        for c in range(NCORES):
            xt = text[c]
            xa = av_feat[c]
            # exact f32 LN statistics on host
            mu_t = xt.mean(-1)
            rinv_t = 1.0 / np.sqrt(xt.var(-1) + LN_EPS)
            mu_a = xa.mean(-1)
            rinv_a = 1.0 / np.sqrt(xa.var(-1) + LN_EPS)

            # text as (t0, t1res) fp8 residual pair, feature-major
            tt = np.ascontiguousarray((xt * rinv_t[:, None]).T.astype(
                np.float32))                                   # [D, NW]
            t0 = tt.astype(fp8)
            t1 = (tt - t0.astype(np.float32)).astype(fp8)
            tT8 = np.concatenate(
                [t0.astype(fp8)[:, None, :], t1[:, None, :]],
                axis=1).reshape(D, 2 * NW)
            aT8 = np.ascontiguousarray(xa.T).astype(fp8)
            mrt = (-mu_t * rinv_t).astype(bf16).reshape(1, NW)
            mra = (-mu_a).astype(bf16).reshape(1, NV)
            # tanh scale: rinv_a * (1/sqrt(dk)) / WSCALE^2 / 2, per kv token
            sca = (rinv_a * (0.125 / (WSCALE * WSCALE) / 2.0)).astype(
                np.float32).reshape(8, 128).T.copy()
            # v drain scale: rinv_a / WSCALE / 2 (tanh = 2*(attn-0.5))
            rva = (rinv_a / (WSCALE * 2.0)).astype(
                np.float32).reshape(8, 128).T.copy()
            # exact 0.5*colsum over kv of v (f32 weights), tiled per fb into
            # the out-psum (qb-repeated) column layout
            a_hat = (xa - mu_a[:, None]) * rinv_a[:, None]
            sv = 0.5 * (a_hat.sum(0) @ wvT_f32)            # [D]
            svt = np.tile(sv.reshape(8, 1, 128), (1, 4, 1)).reshape(
                1, 8 * 512).astype(bf16)
            in_maps.append({
                "aT8": aT8, "tT8": tT8,
                "wq8": wq8, "wk8": wk8, "wv8": wv8,
                "qrow": np.concatenate([mrt, wsq], axis=1),
                "krow": np.concatenate([mra, wsk], axis=1),
                "vrow": np.concatenate([wsv, svt], axis=1),
                "scl": np.concatenate([sca, rva], axis=1),
                "id16": id16,
            })
    else:
        wqT = np.ascontiguousarray((Wq * tn_w[None, :]).T).astype(bf16)
        wkT = np.ascontiguousarray((Wk * an_w[None, :]).T).astype(bf16)
        wvT = np.ascontiguousarray((Wv * an_w[None, :]).T).astype(bf16)
        ident = np.eye(128).astype(bf16)
        nc = _get_program(
            "general_vbias" if np.any(bv_eff) else "general")
        in_maps = [{
            "xt": text[c].astype(bf16),
            "xa": av_feat[c].astype(bf16),
            "wqT": wqT, "wkT": wkT, "wvT": wvT,
            "bq": bq_eff, "bk": bk_eff,
            "bv": bv_eff.astype(bf16).reshape(1, D),
            "ident": ident,
        } for c in range(NCORES)]

    res = run_bass_kernel_spmd(nc, in_maps, core_ids=list(range(NCORES)))
    out = np.stack([res.results[c]["out"] for c in range(NCORES)])
    am = np.stack([np.asarray(res.results[c]["am"], dtype=np.float32)
                   for c in range(NCORES)])
    return out, am
